# revision 5
# baseline (speedup 1.0000x reference)
"""Trainium2 Bass kernel for nn_Adapter (audio conv encoder + cross-attention).

Data-parallel over batch: 16 batches / 8 NeuronCores = 2 per core, no
collectives. All heavy matmuls run in bf16 (1 cycle/row on the PE array);
PSUM accumulation is fp32 throughout, output is fp32.
"""
import sys
sys.path.insert(0, "/opt/trn_rl_repo")

import numpy as np
import ml_dtypes

import concourse.bass as bass
import concourse.mybir as mybir
import concourse.tile as tile
from concourse.bass_utils import run_bass_kernel_spmd

F32 = mybir.dt.float32
BF16 = mybir.dt.bfloat16
AF = mybir.ActivationFunctionType
BF = ml_dtypes.bfloat16

NCORES = 8
B, N, CTX = 16, 4096, 768
BP = B // NCORES            # batches per core
H, D, INNER = 8, 40, 320    # heads, dim_head, inner
AUD = 1024                  # audio feature length
KS, PAD = 17, 8
EPS = 1e-5
SCALE = D ** -0.5
TCH = 512                   # token chunk
NCH = N // TCH              # chunks per batch

# pair -> sim matmul plan: (kp_tile_index, qt_chunk)
SIM_PLAN = [
    [(0, 0)],           # pair 0 (h0,h1): KP01 x qt_ch0
    [(1, 0), (2, 1)],   # pair 1 (h2,h3): KP23a x ch0 + KP23b x ch1
    [(3, 1)],           # pair 2 (h4,h5): KP45 x ch1
    [(4, 1), (5, 2)],   # pair 3 (h6,h7): KP67a x ch1 + KP67b x ch2
]
KP_DEF = [(0, 0), (0, 1), (1, 1), (1, 2), (1, 3), (2, 3)]  # tile -> (chunk, pair)
VM_DEF = [(0, 0), (0, 1), (1, 1), (1, 2), (1, 3), (2, 3)]  # v tiles  (chunk, pair)
AT_V = {0: [0, 1], 1: [1, 2, 3], 2: [3]}                    # chunk -> pairs with v
ME = [128, 128, 64]                                         # e-chunk sizes


def _head_of(e):
    return e // D


def _build_host_consts(inputs):
    c = {}
    w1, b1 = inputs["w1"], inputs["b1"]
    w2, b2 = inputs["w2"], inputs["b2"]
    w3, b3 = inputs["w3"], inputs["b3"]
    c["w1t"] = np.ascontiguousarray(w1[:, 0, :].T).astype(BF)             # [17, 64]
    c["w2t"] = np.ascontiguousarray(w2.transpose(1, 2, 0)).astype(BF)     # [64, 17, 64]
    c["w3t"] = np.ascontiguousarray(w3.transpose(1, 2, 0)).astype(BF)
    c["b1c"] = np.asarray(b1).reshape(64, 1).astype(np.float32)
    c["b2c"] = np.asarray(b2).reshape(64, 1).astype(np.float32)
    c["b3c"] = np.asarray(b3).reshape(64, 1).astype(np.float32)
    c["ln_w"] = np.asarray(inputs["ln_w"]).astype(np.float32)
    c["ln_b"] = np.asarray(inputs["ln_b"]).astype(np.float32)

    wqt = np.zeros((CTX, 384), np.float32)
    wqt[:, :INNER] = np.asarray(inputs["wq"]).T
    c["wqt"] = wqt.astype(BF)
    wkt = np.zeros((AUD, 384), np.float32)
    wkt[:, :INNER] = np.asarray(inputs["wk"]).T
    c["wkt"] = wkt.astype(BF)
    c["wvt"] = np.ascontiguousarray(np.asarray(inputs["wv"]).T).astype(BF)
    wout = np.zeros((384, CTX), np.float32)
    wout[:INNER] = np.asarray(inputs["w_out"]).T
    wout[INNER] = np.asarray(inputs["b_out"])
    c["woutA"] = wout.astype(BF)

    km = np.zeros((128, 6, 128), np.float32)
    for t, (n, p) in enumerate(KP_DEF):
        for r in range(ME[n]):
            h = _head_of(128 * n + r)
            if h == 2 * p:
                km[r, t, 0:64] = 1.0
            elif h == 2 * p + 1:
                km[r, t, 64:128] = 1.0
    c["kmask"] = km.astype(BF)

    vm = np.zeros((128, 6, 128), np.float32)
    for t, (n, p) in enumerate(VM_DEF):
        for col in range(ME[n]):
            h = _head_of(128 * n + col)
            if h == 2 * p:
                vm[0:64, t, col] = 1.0
            elif h == 2 * p + 1:
                vm[64:128, t, col] = 1.0
    c["vmask"] = vm.astype(BF)

    e8 = np.zeros((72, 3, 128), np.float32)
    for n in range(3):
        for r in range(ME[n]):
            e8[64 + _head_of(128 * n + r), n, r] = 1.0
    c["exp8"] = e8.astype(BF)

    c["ident"] = np.eye(64, dtype=np.float32)
    return c


def _build_graph():
    nc = bass.Bass()
    P = {}

    def inp(name, shape, dt):
        P[name] = nc.declare_dram_parameter(name, list(shape), dt, isOutput=False)

    inp("ctx16", (BP, CTX, N), BF16)
    inp("a_im", (BP, KS, AUD), BF16)
    inp("w1t", (KS, 64), BF16)
    inp("w2t", (64, KS, 64), BF16)
    inp("w3t", (64, KS, 64), BF16)
    inp("b1c", (64, 1), F32)
    inp("b2c", (64, 1), F32)
    inp("b3c", (64, 1), F32)
    inp("ln_w", (64, AUD), F32)
    inp("ln_b", (64, AUD), F32)
    inp("wqt", (CTX, 384), BF16)
    inp("wkt", (AUD, 384), BF16)
    inp("wvt", (AUD, INNER), BF16)
    inp("woutA", (384, CTX), BF16)
    inp("kmask", (128, 6, 128), BF16)
    inp("vmask", (128, 6, 128), BF16)
    inp("exp8", (72, 3, 128), BF16)
    inp("ident", (64, 64), F32)
    out_e = nc.declare_dram_parameter("out", [BP, N, CTX], F32, isOutput=True)

    with tile.TileContext(nc) as tc:
        cp = tc.alloc_tile_pool(name="const", bufs=1)
        pp = tc.alloc_tile_pool(name="persist", bufs=2)
        ap = tc.alloc_tile_pool(name="audio", bufs=2)
        aps = tc.alloc_tile_pool(name="aps", bufs=2, space="PSUM")

        # ---- constants ----
        def cload(name, shape, dt, ap_src=None):
            t = cp.tile(list(shape), dt, tag=name)
            nc.sync.dma_start(t[:], ap_src if ap_src is not None else P[name][:])
            return t

        w1t = cload("w1t", (KS, 64), BF16)
        w2t = cload("w2t", (64, KS, 64), BF16)
        w3t = cload("w3t", (64, KS, 64), BF16)
        b1c = cload("b1c", (64, 1), F32)
        b2c = cload("b2c", (64, 1), F32)
        b3c = cload("b3c", (64, 1), F32)
        lnw = cload("ln_w", (64, AUD), F32)
        lnb = cload("ln_b", (64, AUD), F32)
        wqt = cload("wqt", (128, 6, 384), BF16,
                    P["wqt"][:].rearrange("(n p) e -> p n e", p=128))
        wkt = cload("wkt", (128, 8, 384), BF16,
                    P["wkt"][:].rearrange("(n p) e -> p n e", p=128))
        wvt = cload("wvt", (128, 8, INNER), BF16,
                    P["wvt"][:].rearrange("(n p) e -> p n e", p=128))
        woutA = cload("woutA", (128, 3, CTX), BF16,
                      P["woutA"][:].rearrange("(n p) c -> p n c", p=128))
        kmask = cload("kmask", (128, 6, 128), BF16)
        vmask = cload("vmask", (128, 6, 128), BF16)
        exp8 = cload("exp8", (72, 3, 128), BF16)
        ident = cload("ident", (64, 64), F32)

        ones64 = cp.tile([64, 64], BF16, tag="ones64")
        nc.vector.memset(ones64[:], 1.0)

        # denominator-only VP tiles (batch-independent)
        vpd = []
        for p in range(3):
            t = cp.tile([128, 72], BF16, tag=f"vpd{p}")
            nc.vector.memset(t[:], 0.0)
            nc.vector.memset(t[0:64, 64 + 2 * p: 65 + 2 * p], 1.0)
            nc.vector.memset(t[64:128, 65 + 2 * p: 66 + 2 * p], 1.0)
            vpd.append(t)

        # ---- audio encoder per batch ----
        kp_all, vp_all = [], []
        for b in range(BP):
            a_sb = ap.tile([KS, AUD], BF16, tag="a_im")
            nc.sync.dma_start(a_sb[:], P["a_im"][b])

            xb2 = ap.tile([64, AUD + 2 * PAD], BF16, tag="xb2")
            nc.vector.memset(xb2[:, 0:PAD], 0.0)
            nc.vector.memset(xb2[:, AUD + PAD:AUD + 2 * PAD], 0.0)
            for cc in range(2):
                cv1 = aps.tile([64, 512], F32, tag="cv")
                nc.tensor.matmul(cv1[:], w1t[:], a_sb[:, 512 * cc:512 * cc + 512],
                                 start=True, stop=True)
                nc.scalar.activation(xb2[:, PAD + 512 * cc: PAD + 512 * cc + 512],
                                     cv1[:], AF.Gelu, bias=b1c[:])

            x2 = ap.tile([64, AUD], F32, tag="x2")
            stats = ap.tile([64, 4], F32, tag="stats")
            sq_scr = ap.tile([64, 512], F32, tag="sq_scr")
            for cc in range(2):
                cv2 = aps.tile([64, 512], F32, tag="cv")
                for k in range(KS):
                    nc.tensor.matmul(cv2[:], w2t[:, k, :],
                                     xb2[:, k + 512 * cc: k + 512 * cc + 512],
                                     start=(k == 0), stop=(k == KS - 1))
                nc.vector.tensor_scalar(
                    out=x2[:, 512 * cc:512 * cc + 512], in0=cv2[:],
                    scalar1=b2c[:], scalar2=0.0, op0=mybir.AluOpType.add,
                    op1=mybir.AluOpType.add, accum_out=stats[:, cc:cc + 1])
                nc.scalar.activation(sq_scr[:], x2[:, 512 * cc:512 * cc + 512],
                                     AF.Square, accum_out=stats[:, 2 + cc:3 + cc])

            tot16 = ap.tile([64, 2], BF16, tag="tot16")
            nc.vector.tensor_add(tot16[:, 0:1], stats[:, 0:1], stats[:, 1:2])
            nc.vector.tensor_add(tot16[:, 1:2], stats[:, 2:3], stats[:, 3:4])
            totp = aps.tile([64, 64], F32, tag="pt")
            nc.tensor.matmul(totp[:, 0:2], ones64[:], tot16[:], start=True, stop=True)

            mu = ap.tile([64, 1], F32, tag="mu")
            msq = ap.tile([64, 1], F32, tag="msq")
            var = ap.tile([64, 1], F32, tag="var")
            sd = ap.tile([64, 1], F32, tag="sd")
            rstd = ap.tile([64, 1], F32, tag="rstd")
            nmr = ap.tile([64, 1], F32, tag="nmr")
            inv_n = 1.0 / (64 * AUD)
            nc.vector.tensor_scalar_mul(mu[:], totp[:, 0:1], inv_n)
            nc.vector.tensor_scalar_mul(msq[:], totp[:, 1:2], inv_n)
            nc.vector.tensor_mul(var[:], mu[:], mu[:])
            nc.vector.tensor_sub(var[:], msq[:], var[:])
            nc.vector.tensor_scalar_add(var[:], var[:], EPS)
            nc.scalar.activation(sd[:], var[:], AF.Sqrt)
            nc.vector.reciprocal(rstd[:], sd[:])
            nc.vector.tensor_mul(nmr[:], mu[:], rstd[:])
            nc.vector.tensor_scalar_mul(nmr[:], nmr[:], -1.0)

            t1 = ap.tile([64, AUD], F32, tag="t1")
            t2 = ap.tile([64, AUD], F32, tag="t2")
            xb3 = ap.tile([64, AUD + 2 * PAD], BF16, tag="xb3")
            nc.vector.memset(xb3[:, 0:PAD], 0.0)
            nc.vector.memset(xb3[:, AUD + PAD:AUD + 2 * PAD], 0.0)
            nc.vector.tensor_scalar(out=t1[:], in0=x2[:], scalar1=rstd[:],
                                    scalar2=nmr[:], op0=mybir.AluOpType.mult,
                                    op1=mybir.AluOpType.add)
            nc.vector.tensor_mul(t2[:], t1[:], lnw[:])
            nc.vector.tensor_add(xb3[:, PAD:PAD + AUD], t2[:], lnb[:])

            x_sb = ap.tile([64, AUD], F32, tag="x_sb")
            for cc in range(2):
                cv3 = aps.tile([64, 512], F32, tag="cv")
                for k in range(KS):
                    nc.tensor.matmul(cv3[:], w3t[:, k, :],
                                     xb3[:, k + 512 * cc: k + 512 * cc + 512],
                                     start=(k == 0), stop=(k == KS - 1))
                nc.vector.tensor_scalar(
                    out=x_sb[:, 512 * cc:512 * cc + 512], in0=cv3[:],
                    scalar1=b3c[:], scalar2=0.0, op0=mybir.AluOpType.add,
                    op1=mybir.AluOpType.add)

            xt = pp.tile([128, 8, 64], BF16, tag="xt")
            for f in range(8):
                pt = aps.tile([128, 64], F32, tag="pt")
                nc.tensor.transpose(pt[:], x_sb[:, 128 * f:128 * f + 128], ident[:])
                nc.vector.tensor_copy(xt[:, f, :], pt[:])

            kt = pp.tile([128, 3, 64], BF16, tag="kt")
            for m in range(3):
                ktp = aps.tile([128, 64], F32, tag="pt")
                for aj in range(8):
                    nc.tensor.matmul(ktp[:], wkt[:, aj, 128 * m:128 * m + 128],
                                     xt[:, aj, :], start=(aj == 0), stop=(aj == 7))
                nc.vector.tensor_copy(kt[:, m, :], ktp[:])

            v2p = aps.tile([128, INNER], F32, tag="v2p")
            for half in range(2):
                for aj in range(8):
                    nc.tensor.matmul(v2p[64 * half:64 * half + 64, :],
                                     xt[:, aj, :], wvt[:, aj, :],
                                     start=(aj == 0), stop=(aj == 7))
            v2 = pp.tile([128, INNER], BF16, tag="v2")
            nc.scalar.activation(v2[:], v2p[:], AF.Copy)

            kps = []
            for t, (n, p) in enumerate(KP_DEF):
                kpt = pp.tile([128, 128], BF16, tag=f"kp{t}")
                nc.vector.tensor_mul(
                    kpt[:].rearrange("p (a b) -> p a b", a=2),
                    kt[:, n:n + 1, :].broadcast_to([128, 2, 64]),
                    kmask[:, t, :].rearrange("p (a b) -> p a b", a=2))
                kps.append(kpt)
            kp_all.append(kps)

            vps = {}
            for t, (n, p) in enumerate(VM_DEF):
                w = 72 if n == 2 else ME[n]
                vpt = pp.tile([128, w], BF16, tag=f"vp{t}")
                nc.vector.tensor_mul(vpt[:, 0:ME[n]],
                                     v2[:, 128 * n:128 * n + ME[n]],
                                     vmask[:, t, 0:ME[n]])
                if n == 2:
                    nc.vector.memset(vpt[:, 64:72], 0.0)
                    nc.vector.memset(vpt[0:64, 64 + 2 * p:65 + 2 * p], 1.0)
                    nc.vector.memset(vpt[64:128, 65 + 2 * p:66 + 2 * p], 1.0)
                vps[(n, p)] = vpt
            vp_all.append(vps)

        ap.release()
        aps.release()

        # ---- main attention loop ----
        cinp = tc.alloc_tile_pool(name="cinp", bufs=3)
        esp = tc.alloc_tile_pool(name="esp", bufs=6)
        mp = tc.alloc_tile_pool(name="mp", bufs=2)
        ofp = tc.alloc_tile_pool(name="ofp", bufs=3)
        mps = tc.alloc_tile_pool(name="mps", bufs=2, space="PSUM")

        for b in range(BP):
            kps = kp_all[b]
            vps = vp_all[b]
            ctx_ap = P["ctx16"][b].rearrange("(n p) t -> p n t", p=128)
            for c in range(NCH):
                cin = cinp.tile([128, 6, TCH], BF16, tag="cin")
                nc.sync.dma_start(cin[:], ctx_ap[:, :, TCH * c:TCH * c + TCH])

                qt = mp.tile([128, 3, TCH], BF16, tag="qt")
                for m in range(3):
                    qp = mps.tile([128, TCH], F32, tag="qp")
                    for n6 in range(6):
                        nc.tensor.matmul(qp[:], wqt[:, n6, 128 * m:128 * m + 128],
                                         cin[:, n6, :], start=(n6 == 0),
                                         stop=(n6 == 5))
                    nc.vector.tensor_copy(qt[:, m, :], qp[:])

                es = []
                for p in range(4):
                    sp = mps.tile([128, TCH], F32, tag="sp")
                    plan = SIM_PLAN[p]
                    for i, (kpi, qch) in enumerate(plan):
                        nc.tensor.matmul(sp[:], kps[kpi][:], qt[:, qch, :],
                                         start=(i == 0), stop=(i == len(plan) - 1))
                    e = esp.tile([128, TCH], BF16, tag="es")
                    nc.scalar.activation(e[:], sp[:], AF.Exp, scale=SCALE)
                    es.append(e)

                at_sb = mp.tile([128, 3, TCH], BF16, tag="at_sb")
                nc.vector.memset(at_sb[64:65, 2, :], 1.0)

                # chunk n2 first: carries denominators in rows 64:72
                at2 = mps.tile([128, TCH], F32, tag="at")
                n2_ops = [(vpd[0], 0), (vpd[1], 1), (vpd[2], 2), (vps[(2, 3)], 3)]
                for i, (vpt, p) in enumerate(n2_ops):
                    nc.tensor.matmul(at2[0:72, :], vpt[:], es[p][:],
                                     start=(i == 0), stop=(i == 3))
                rec = mp.tile([72, TCH], F32, tag="rec")
                rec16 = mp.tile([72, TCH], BF16, tag="rec16")
                nc.vector.reciprocal(rec[64:72, :], at2[64:72, :])
                nc.vector.tensor_copy(rec16[64:72, :], rec[64:72, :])

                def normalize(n, at_ps):
                    brp = mps.tile([128, TCH], F32, tag="ob")
                    nc.tensor.matmul(brp[0:ME[n], :], exp8[64:72, n, 0:ME[n]],
                                     rec16[64:72, :], start=True, stop=True)
                    brs = mp.tile([128, TCH], F32, tag="brs")
                    nc.vector.tensor_copy(brs[0:ME[n], :], brp[0:ME[n], :])
                    nc.vector.tensor_mul(at_sb[0:ME[n], n, :],
                                         at_ps[0:ME[n], :], brs[0:ME[n], :])

                for n in (0, 1):
                    a = mps.tile([128, TCH], F32, tag="at")
                    prs = AT_V[n]
                    for i, p in enumerate(prs):
                        nc.tensor.matmul(a[0:ME[n], :], vps[(n, p)][:, 0:ME[n]],
                                         es[p][:], start=(i == 0),
                                         stop=(i == len(prs) - 1))
                    normalize(n, a)
                normalize(2, at2)

                for tt in range(4):
                    of = ofp.tile([128, CTX], F32, tag="of")
                    for ci, (c0, cw) in enumerate(((0, 512), (512, 256))):
                        op = mps.tile([128, 512], F32, tag="ob")
                        for n in range(3):
                            rows = 65 if n == 2 else 128
                            nc.tensor.matmul(
                                op[:, 0:cw],
                                at_sb[0:rows, n, 128 * tt:128 * tt + 128],
                                woutA[0:rows, n, c0:c0 + cw],
                                start=(n == 0), stop=(n == 2))
                        if ci == 0:
                            nc.scalar.activation(of[:, c0:c0 + cw], op[:, 0:cw],
                                                 AF.Copy)
                        else:
                            nc.vector.tensor_copy(of[:, c0:c0 + cw], op[:, 0:cw])
                    nc.sync.dma_start(
                        out_e[b, TCH * c + 128 * tt: TCH * c + 128 * tt + 128, :],
                        of[:])

        mps.release()
        ofp.release()
        mp.release()
        esp.release()
        cinp.release()
        pp.release()
        cp.release()

    split_waits(nc)
    return nc


def split_waits(nc, max_waits=1):
    """neuronxcc walrus accepts at most one attached sync wait per
    instruction; hoist extras onto standalone event-semaphore waits."""
    n_new = 0
    for f in nc.m.functions:
        for blk in f.blocks:
            new = []
            changed = False
            for inst in blk.instructions:
                si = inst.sync_info
                ow = list(si.on_wait) if (si is not None and si.on_wait) else []
                if len(ow) > max_waits:
                    for w in ow[:-max_waits]:
                        ev = mybir.InstEventSemaphore(
                            name=f"I-waitsplit-{n_new}", ins=[], outs=[])
                        ev.engine = inst.engine
                        ev.sync_info = mybir.SyncInfo(on_wait=[w], on_update=[])
                        nc.register_instruction(ev)
                        new.append(ev)
                        n_new += 1
                    inst.sync_info = mybir.SyncInfo(
                        on_wait=ow[-max_waits:], on_update=list(si.on_update))
                    changed = True
                new.append(inst)
            if changed:
                blk.instructions = new


_GRAPH = None


def kernel(**inputs):
    global _GRAPH
    if _GRAPH is None:
        _GRAPH = _build_graph()
    nc = _GRAPH

    consts = _build_host_consts(inputs)
    ctx = np.asarray(inputs["context"])           # [16, 4096, 768] f32
    audio = np.asarray(inputs["audio_context"])   # [16, 1, 1024] f32

    ctx16 = np.ascontiguousarray(ctx.transpose(0, 2, 1)).astype(BF)
    apad = np.zeros((B, AUD + 2 * PAD), np.float32)
    apad[:, PAD:PAD + AUD] = audio[:, 0, :]
    a_im = np.empty((B, KS, AUD), np.float32)
    for k in range(KS):
        a_im[:, k, :] = apad[:, k:k + AUD]
    a_im = a_im.astype(BF)

    in_maps = []
    for core in range(NCORES):
        m = dict(consts)
        s = slice(core * BP, (core + 1) * BP)
        m["ctx16"] = ctx16[s]
        m["a_im"] = a_im[s]
        in_maps.append(m)

    res = run_bass_kernel_spmd(nc, in_maps, list(range(NCORES)))
    out = np.concatenate([res.results[i]["out"] for i in range(NCORES)], axis=0)
    return out.astype(np.float32)
